# revision 1
# baseline (speedup 1.0000x reference)
"""Causal no-head self-attention with RoPE on 8 Trainium2 NeuronCores.

Sharding: 8 cores = 4 batches x 2 balanced causal query-sets (zigzag):
  core (b, 0): query blocks [0:512) and [1536:2048)   (kt-structure 8, 16)
  core (b, 1): query blocks [512:1024) and [1024:1536) (kt-structure 8, 16)
All cores run ONE identical Bass program; per-core differences (which
queries, causal masks, RoPE angles) are carried in the input data.

Device layouts (transposed, channel-on-partition):
  QT/KT: [d_k, seq] with d_k de-interleaved (even feats rows 0:512, odd
  512:1024) so RoPE is a contiguous-partition-block rotation. The same
  permutation is applied to Wq/Wk rows on host (scores are invariant).
  V: natural [seq, d_v]. All matmuls in float32r (full PE rate at N>=256).
"""

import numpy as np
import sys

for _p in ("/opt/trn_rl_repo",):
    if _p not in sys.path:
        sys.path.insert(0, _p)

import concourse.bass as bass
import concourse.bacc as bacc
import concourse.mybir as mybir
from concourse.tile import TileContext
from concourse.bass_utils import run_bass_kernel_spmd

B, S, D = 4, 2048, 1024
THETA = 10000.0
P = 128
NT = D // P          # 8 partition-tiles over a 1024 dim
QB = 512             # query block width (2 blocks per core)
NKT_A, NKT_B = 8, 16  # kt visits for q-block A / B (uniform structure)
F32 = mybir.dt.float32
F32R = mybir.dt.float32r
BF16 = mybir.dt.bfloat16
SCALE = 1.0 / 32.0   # 1/sqrt(D)


def _build_program():
    nc = bacc.Bacc("TRN2", num_swdge_queues=4)
    inp = {}
    def din(name, shape, dt):
        inp[name] = nc.dram_tensor(name, shape, dt, kind="ExternalInput")
    din("xT", [D, S], F32R)
    din("xTq", [D, 2 * QB], F32R)
    din("WqT", [D, D], F32R)
    din("WkT", [D, D], F32R)
    din("WvT", [D, D], F32R)
    din("WoT", [D, D], F32R)
    din("cosK", [D // 2, S], F32)
    din("sinK", [D // 2, S], F32)
    din("cosQ", [D // 2, 2 * QB], F32)
    din("sinQ", [D // 2, 2 * QB], F32)
    din("ones_col", [P, 1], F32R)
    din("ones_row", [1, P], F32)
    din("maskA", [P, NKT_A, QB], BF16)
    din("maskB", [P, NKT_B, QB], BF16)
    outT = nc.dram_tensor("outT", [P, NT, 2 * QB], F32, kind="ExternalOutput")

    xT_r = inp["xT"].rearrange("(t p) s -> p t s", p=P)
    xTq_r = inp["xTq"].rearrange("(t p) s -> p t s", p=P)
    WqT_r = inp["WqT"].rearrange("(t p) o -> p t o", p=P)
    WkT_r = inp["WkT"].rearrange("(t p) o -> p t o", p=P)
    WvT_r = inp["WvT"].rearrange("(t p) o -> p t o", p=P)
    WoT_r = inp["WoT"].rearrange("(t p) o -> p t o", p=P)
    cosK_r = inp["cosK"].rearrange("(t p) s -> p t s", p=P)
    sinK_r = inp["sinK"].rearrange("(t p) s -> p t s", p=P)
    cosQ_r = inp["cosQ"].rearrange("(t p) s -> p t s", p=P)
    sinQ_r = inp["sinQ"].rearrange("(t p) s -> p t s", p=P)

    from contextlib import ExitStack
    with TileContext(nc) as tc:
        with ExitStack() as ctx:
            pool = lambda *a, **kw: ctx.enter_context(tc.tile_pool(*a, **kw))
            dpool = pool(name="dram", bufs=1, space="DRAM")
            wres = pool(name="wres", bufs=1)        # resident weight (32KB)
            smp = pool(name="small", bufs=1)
            psA = pool(name="psA", bufs=4, space="PSUM")
            psB = pool(name="psB", bufs=2, space="PSUM")
            psS = pool(name="psS", bufs=1, space="PSUM")

            # Per-seq-block spill tiles: a kt-tile read only waits on its
            # own block's spill write, so attention overlaps late KV phase.
            KTd = [dpool.tile([QB // P, P, NT, P], F32R, name=f"ktd{i}") for i in range(S // QB)]
            Vd = [dpool.tile([P, QB // P, D], F32R, name=f"vd{i}") for i in range(S // QB)]

            ones_col = smp.tile([P, 1], F32R, tag="onescol")
            nc.sync.dma_start(ones_col[:], inp["ones_col"][:])
            ones_row = smp.tile([1, P], F32, tag="onesrow")
            nc.sync.dma_start(ones_row[:], inp["ones_row"][:])

            def rope_block(dst, src, cos_t, sin_t, tmpp):
                # dst/src: [P, NT, QB]; rows 0:NT/2 = even feats, NT/2:NT = odd
                h = NT // 2
                e, o = src[:, 0:h, :], src[:, h:NT, :]
                c, s = cos_t[:, :, :], sin_t[:, :, :]
                t1 = tmpp.tile([P, h, QB], F32, tag="ropetmp", name="t1")
                nc.vector.tensor_mul(out=dst[:, 0:h, :], in0=e, in1=c)
                nc.vector.tensor_mul(out=t1[:], in0=o, in1=s)
                nc.vector.tensor_tensor(dst[:, 0:h, :], dst[:, 0:h, :],
                                        t1[:], mybir.AluOpType.subtract)
                t2 = tmpp.tile([P, h, QB], F32, tag="ropetmp", name="t2")
                nc.vector.tensor_mul(out=dst[:, h:NT, :], in0=o, in1=c)
                nc.vector.tensor_mul(out=t2[:], in0=e, in1=s)
                nc.vector.tensor_tensor(dst[:, h:NT, :], dst[:, h:NT, :],
                                        t2[:], mybir.AluOpType.add)

            # ---------- Phase 1: Q^T projection + RoPE (both q blocks) -----------
            qtp = pool(name="qt", bufs=1)           # resident Q^T (32KB)
            QT = qtp.tile([P, NT, 2 * QB], F32R, tag="qt")
            with ExitStack() as p1:
                pp = lambda *a, **kw: p1.enter_context(tc.tile_pool(*a, **kw))
                xbp = pp(name="xb1", bufs=2)
                rawp = pp(name="raw1", bufs=1)
                rotp = pp(name="rot1", bufs=1)
                csp = pp(name="cs1", bufs=1)
                tmpp = pp(name="tmp1", bufs=1)

                WqRes = wres.tile([P, NT, D], F32R, tag="w")
                for t in range(NT):
                    nc.sync.dma_start(WqRes[:, t, :], WqT_r[:, t, :])
                for qb in range(2):
                    sl = slice(qb * QB, (qb + 1) * QB)
                    xq = xbp.tile([P, NT, QB], F32R, tag="xb")
                    for t in range(NT):
                        nc.sync.dma_start(xq[:, t, :], xTq_r[:, t, sl])
                    qraw = rawp.tile([P, NT, QB], F32, tag="raw")
                    for t_out in range(NT):
                        ps = psB.tile([P, QB], F32, tag="psB")
                        for dt_ in range(NT):
                            nc.tensor.matmul(ps[:], WqRes[:, dt_, t_out * P:(t_out + 1) * P],
                                             xq[:, dt_, :], start=(dt_ == 0), stop=(dt_ == NT - 1))
                        nc.scalar.copy(qraw[:, t_out, :], ps[:])
                    cq = csp.tile([P, NT // 2, QB], F32, tag="cs_c")
                    nc.gpsimd.dma_start(cq[:], cosQ_r[:, :, sl])
                    sq = csp.tile([P, NT // 2, QB], F32, tag="cs_s")
                    nc.gpsimd.dma_start(sq[:], sinQ_r[:, :, sl])
                    qrot = rotp.tile([P, NT, QB], F32R, tag="rot")
                    rope_block(qrot, qraw, cq, sq, tmpp)
                    nc.vector.tensor_copy(QT[:, :, sl], qrot[:])

            # ---------- Phase 0: K^T and V projection (fused over seq blocks) ----
            with ExitStack() as p0:
                pp = lambda *a, **kw: p0.enter_context(tc.tile_pool(*a, **kw))
                xbp = pp(name="xb0", bufs=2)
                rawp = pp(name="raw0", bufs=1)
                rotp = pp(name="rot0", bufs=1)
                csp = pp(name="cs0", bufs=1)
                tmpp = pp(name="tmp0", bufs=1)
                wres2 = pp(name="wres2", bufs=1)
                vbp = pp(name="vb", bufs=1)

                WkRes = wres.tile([P, NT, D], F32R, tag="w")
                for t in range(NT):
                    nc.sync.dma_start(WkRes[:, t, :], WkT_r[:, t, :])
                WvRes = wres2.tile([P, NT, D], F32R, tag="w2")
                for t in range(NT):
                    nc.sync.dma_start(WvRes[:, t, :], WvT_r[:, t, :])

                for sb in range(S // QB):           # 4 seq blocks of 512
                    sl = slice(sb * QB, (sb + 1) * QB)
                    xb = xbp.tile([P, NT, QB], F32R, tag="xb")
                    for t in range(NT):
                        nc.sync.dma_start(xb[:, t, :], xT_r[:, t, sl])
                    # K^T block: out rows t_out, cols = keys in this block
                    kraw = rawp.tile([P, NT, QB], F32, tag="raw")
                    for t_out in range(NT):
                        ps = psB.tile([P, QB], F32, tag="psB")
                        for dt_ in range(NT):
                            nc.tensor.matmul(ps[:], WkRes[:, dt_, t_out * P:(t_out + 1) * P],
                                             xb[:, dt_, :], start=(dt_ == 0), stop=(dt_ == NT - 1))
                        nc.scalar.copy(kraw[:, t_out, :], ps[:])
                    ck = csp.tile([P, NT // 2, QB], F32, tag="cs_c")
                    nc.gpsimd.dma_start(ck[:], cosK_r[:, :, sl])
                    sk = csp.tile([P, NT // 2, QB], F32, tag="cs_s")
                    nc.gpsimd.dma_start(sk[:], sinK_r[:, :, sl])
                    krot = rotp.tile([P, NT, QB], F32R, tag="rot")
                    rope_block(krot, kraw, ck, sk, tmpp)
                    for ks in range(QB // P):
                        nc.sync.dma_start(KTd[sb][ks], krot[:, :, ks * P:(ks + 1) * P])
                    # V rows for this block: out[seq-chunk, dv]
                    vb = vbp.tile([P, QB // P, D], F32R, tag="vb")
                    for dvb in range(2):
                        for sk_ in range(QB // P):
                            ps = psA.tile([P, QB], F32, tag="psA")
                            for dt_ in range(NT):
                                nc.tensor.matmul(ps[:], xb[:, dt_, sk_ * P:(sk_ + 1) * P],
                                                 WvRes[:, dt_, dvb * QB:(dvb + 1) * QB],
                                                 start=(dt_ == 0), stop=(dt_ == NT - 1))
                            nc.scalar.copy(vb[:, sk_, dvb * QB:(dvb + 1) * QB], ps[:])
                    nc.sync.dma_start(Vd[sb][:], vb[:])

            # ---------- Phase 2: attention + output projection per q block ------
            ptp = pool(name="pt", bufs=1)           # exp(scores)
            atp = pool(name="at", bufs=1)           # attnT
            ktsp = pool(name="kts", bufs=4)         # streamed KT tiles
            vtsp = pool(name="vts", bufs=6)         # streamed V tiles
            mskp = pool(name="msk", bufs=1)
            outp = pool(name="outb", bufs=1)

            maskA_t = mskp.tile([P, NKT_A, QB], BF16, tag="mA")
            nc.gpsimd.dma_start(maskA_t[:], inp["maskA"][:])
            maskB_t = mskp.tile([P, NKT_B, QB], BF16, tag="mB")
            nc.gpsimd.dma_start(maskB_t[:], inp["maskB"][:])

            WoRes = wres.tile([P, NT, D], F32R, tag="w")
            for t in range(NT):
                nc.sync.dma_start(WoRes[:, t, :], WoT_r[:, t, :])

            for qb, nkt, mask_t in ((0, NKT_A, maskA_t), (1, NKT_B, maskB_t)):
                sl = slice(qb * QB, (qb + 1) * QB)
                PT = ptp.tile([P, NKT_B, QB], F32R, tag="pt")
                sums = psS.tile([1, QB], F32, tag="psS")
                # wave-0 PV accumulates inside the scores loop (psA is idle
                # there), overlapping half of PV with the scores stream.
                attnT = atp.tile([P, NT, QB], F32R, tag="at")
                psvs0 = [psA.tile([P, QB], F32, tag="psA", name=f"psv0{_j}") for _j in range(4)]
                for kt in range(nkt):
                    ktile = ktsp.tile([P, NT, P], F32R, tag="kts")
                    nc.sync.dma_start(ktile[:], KTd[kt // (QB // P)][kt % (QB // P)])
                    ps = psB.tile([P, QB], F32, tag="psB")
                    for dt_ in range(NT):
                        nc.tensor.matmul(ps[:], ktile[:, dt_, :], QT[:, dt_, sl],
                                         start=(dt_ == 0), stop=(dt_ == NT - 1))
                    nc.scalar.activation(PT[:, kt, :], ps[:],
                                         mybir.ActivationFunctionType.Exp, scale=SCALE)
                    nc.vector.tensor_mul(out=PT[:, kt, :], in0=PT[:, kt, :],
                                         in1=mask_t[:, kt, :])
                    vtile = vtsp.tile([P, QB], F32R, tag="vts")
                    nc.sync.dma_start(vtile[:], Vd[kt // (QB // P)][:, kt % (QB // P), 0:QB])
                    for j in range(4):
                        nc.tensor.matmul(psvs0[j][:], vtile[:, j * P:(j + 1) * P],
                                         PT[:, kt, :], start=(kt == 0), stop=(kt == nkt - 1))
                for kt in range(nkt):
                    nc.tensor.matmul(sums[:], ones_col[:], PT[:, kt, :],
                                     start=(kt == 0), stop=(kt == nkt - 1))
                recip = smp.tile([1, QB], F32, tag="recip")
                nc.vector.reciprocal(recip[:], sums[:])
                bc_ps = psB.tile([P, QB], F32, tag="psB")
                nc.tensor.matmul(bc_ps[:], ones_row[:], recip[:], start=True, stop=True)
                bc = smp.tile([P, QB], F32, tag="bc")
                nc.scalar.copy(bc[:], bc_ps[:])

                for j in range(4):
                    nc.vector.tensor_mul(out=attnT[:, j, :],
                                         in0=psvs0[j][:], in1=bc[:])
                psvs1 = [psA.tile([P, QB], F32, tag="psA", name=f"psv1{_j}") for _j in range(4)]
                for kt in range(nkt):
                    vtile = vtsp.tile([P, QB], F32R, tag="vts")
                    nc.sync.dma_start(vtile[:], Vd[kt // (QB // P)][:, kt % (QB // P), QB:2 * QB])
                    for j in range(4):
                        nc.tensor.matmul(psvs1[j][:], vtile[:, j * P:(j + 1) * P],
                                         PT[:, kt, :], start=(kt == 0), stop=(kt == nkt - 1))
                for j in range(4):
                    nc.vector.tensor_mul(out=attnT[:, 4 + j, :],
                                         in0=psvs1[j][:], in1=bc[:])

                ob = outp.tile([P, NT, QB], F32, tag="outb")
                for oc in range(NT):
                    ps = psB.tile([P, QB], F32, tag="psB")
                    for dt_ in range(NT):
                        nc.tensor.matmul(ps[:], WoRes[:, dt_, oc * P:(oc + 1) * P],
                                         attnT[:, dt_, :], start=(dt_ == 0), stop=(dt_ == NT - 1))
                    nc.scalar.copy(ob[:, oc, :], ps[:])
                nc.sync.dma_start(outT[:, :, sl], ob[:])

    nc.finalize()
    return nc


def _host_inputs(x, Wq, Wk, Wv, Wo, token_positions):
    perm = np.concatenate([np.arange(0, D, 2), np.arange(1, D, 2)])
    WqTp = np.ascontiguousarray(Wq[perm].T.astype(np.float32))
    WkTp = np.ascontiguousarray(Wk[perm].T.astype(np.float32))
    WvT = np.ascontiguousarray(Wv.T.astype(np.float32))
    WoT = np.ascontiguousarray(Wo.T.astype(np.float32))
    inv_freq = (1.0 / (np.float32(THETA) **
                       (np.arange(0, D, 2, dtype=np.float32) / np.float32(D))))
    inv_freq = inv_freq.astype(np.float32)

    in_maps, metas = [], []
    for b in range(B):
        xT = np.ascontiguousarray(x[b].T.astype(np.float32))   # [D, S]
        pos = token_positions[b].astype(np.float32)
        ang = (pos[None, :] * inv_freq[:, None]).astype(np.float32)  # [D/2, S]
        cosF = np.cos(ang).astype(np.float32)
        sinF = np.sin(ang).astype(np.float32)
        for h in range(2):
            if h == 0:
                qcols = np.r_[0:QB, 3 * QB:4 * QB]
                q0s = (0, 3 * QB)          # global start of q-block A, B
            else:
                qcols = np.r_[QB:2 * QB, 2 * QB:3 * QB]
                q0s = (QB, 2 * QB)
            xTq = np.ascontiguousarray(xT[:, qcols])
            cosQ = np.ascontiguousarray(cosF[:, qcols])
            sinQ = np.ascontiguousarray(sinF[:, qcols])
            masks = []
            for (q0, nkt) in zip(q0s, (NKT_A, NKT_B)):
                m = np.zeros((P, nkt, QB), dtype=np.float32)
                for kt in range(nkt):
                    kbase = kt * P
                    # valid iff (q0 + q) >= (kbase + k)
                    q_glob = q0 + np.arange(QB)
                    k_glob = kbase + np.arange(P)
                    m[:, kt, :] = (q_glob[None, :] >= k_glob[:, None])
                masks.append(m)
            import ml_dtypes
            maskA = masks[0].astype(ml_dtypes.bfloat16)
            maskB = masks[1].astype(ml_dtypes.bfloat16)
            in_maps.append({
                "ones_col": np.ones((P, 1), np.float32),
                "ones_row": np.ones((1, P), np.float32),
                "xT": xT, "xTq": xTq,
                "WqT": WqTp, "WkT": WkTp, "WvT": WvT, "WoT": WoT,
                "cosK": cosF, "sinK": sinF, "cosQ": cosQ, "sinQ": sinQ,
                "maskA": maskA, "maskB": maskB,
            })
            metas.append((b, qcols))
    return in_maps, metas


_NC_CACHE = {}


def kernel(x, Wq, Wk, Wv, Wo, token_positions):
    x = np.asarray(x); token_positions = np.asarray(token_positions)
    if "nc" not in _NC_CACHE:
        _NC_CACHE["nc"] = _build_program()
    nc = _NC_CACHE["nc"]
    in_maps, metas = _host_inputs(np.asarray(x), np.asarray(Wq), np.asarray(Wk),
                                  np.asarray(Wv), np.asarray(Wo), token_positions)
    res = run_bass_kernel_spmd(nc, in_maps, core_ids=list(range(8)))
    out = np.empty((B, S, D), dtype=np.float32)
    for (b, qcols), r in zip(metas, res.results):
        oT = r["outT"]                       # [P, NT, 2*QB]
        o = np.transpose(oT, (2, 1, 0)).reshape(2 * QB, D)
        out[b, qcols, :] = o
    return out



# revision 8
# speedup vs baseline: 1.2462x; 1.2462x over previous
"""Causal no-head self-attention with RoPE on 8 Trainium2 NeuronCores.

Sharding: 8 cores = 4 batches x 2 query-sets of four 256-query blocks.
Per-core slots s=0..3 run 4(s+1) key-tile visits (128 keys each); block
assignment (h=0: blocks {1,3,4,7}, h=1: {0,2,5,6}) makes the same
(4,8,12,16) visit structure causally sufficient on every core, so all
cores run ONE identical Bass program; per-core differences (which
queries, causal masks, RoPE angles) are carried in the input data.

Everything is bf16 (PE full rate, half DMA/SBUF of fp32) with fp32 PSUM
accumulation. K^T and V live entirely in SBUF (no DRAM spill). Device
layouts are transposed, channel-on-partition: QT/KT [d_k, q|s] with d_k
de-interleaved (even feats rows 0:512, odd 512:1024) so RoPE is a
contiguous-partition-block rotation; the same permutation is applied to
Wq/Wk output columns on host (scores are invariant). V natural [s, d_v].
"""

import numpy as np
import sys

for _p in ("/opt/trn_rl_repo",):
    if _p not in sys.path:
        sys.path.insert(0, _p)

import concourse.bass as bass
import concourse.bacc as bacc
import concourse.mybir as mybir
from concourse.tile import TileContext
from concourse.bass_utils import run_bass_kernel_spmd

B, S, D = 4, 2048, 1024
THETA = 10000.0
P = 128
NT = D // P          # 8 partition-tiles over the 1024 dim
SB = 512             # seq block width for K/V projection (4 blocks)
QB = 256             # query slot width (4 slots per core)
NQ = 1024            # queries per core
F32 = mybir.dt.float32
F32R = mybir.dt.float32r
BF16 = mybir.dt.bfloat16
SCALE = 1.0 / 32.0   # 1/sqrt(D)
BLOCKS = [[1, 3, 4, 7], [0, 2, 5, 6]]   # 256-query blocks per core half
NVIS = [4, 8, 12, 16]                    # kt visits per slot (same all cores)


def _build_program():
    nc = bacc.Bacc("TRN2", num_swdge_queues=4)
    inp = {}
    def din(name, shape, dt):
        inp[name] = nc.dram_tensor(name, shape, dt, kind="ExternalInput")
    din("xT", [D, S], BF16)
    din("xTq", [D, NQ], BF16)
    din("WqT", [D, D], BF16)
    din("WkT", [D, D], BF16)
    din("WvT", [D, D], BF16)
    din("WoT", [D, D], BF16)
    din("cosK", [D // 2, S], BF16)
    din("sinK", [D // 2, S], BF16)
    din("cosQ", [D // 2, NQ], BF16)
    din("sinQ", [D // 2, NQ], BF16)
    din("masks", [P, 16, QB], BF16)
    din("ones_col", [P, 1], F32R)
    din("ones_row", [1, P], F32)
    outT = nc.dram_tensor("outT", [P, NT, NQ], BF16, kind="ExternalOutput")

    xT_r = inp["xT"].rearrange("(t p) s -> p t s", p=P)
    xTq_r = inp["xTq"].rearrange("(t p) s -> p t s", p=P)
    WqT_r = inp["WqT"].rearrange("(t p) o -> p t o", p=P)
    WkT_r = inp["WkT"].rearrange("(t p) o -> p t o", p=P)
    WvT_r = inp["WvT"].rearrange("(t p) o -> p t o", p=P)
    WoT_r = inp["WoT"].rearrange("(t p) o -> p t o", p=P)
    cosK_r = inp["cosK"].rearrange("(t p) s -> p t s", p=P)
    sinK_r = inp["sinK"].rearrange("(t p) s -> p t s", p=P)
    cosQ_r = inp["cosQ"].rearrange("(t p) s -> p t s", p=P)
    sinQ_r = inp["sinQ"].rearrange("(t p) s -> p t s", p=P)

    from contextlib import ExitStack
    with TileContext(nc) as tc:
        with ExitStack() as ctx:
            pool = lambda *a, **kw: ctx.enter_context(tc.tile_pool(*a, **kw))
            res = pool(name="res", bufs=1)          # big residents
            wres = pool(name="wres", bufs=1)        # weights
            smp = pool(name="small", bufs=1)
            xbp = pool(name="xb", bufs=2)
            csp = pool(name="cs", bufs=1)
            rawp = pool(name="raw", bufs=1)
            tmpp = pool(name="tmp", bufs=1)
            ptp = pool(name="pt", bufs=1)
            sap = pool(name="sa", bufs=1)
            bcp = pool(name="bc", bufs=2)
            obp = pool(name="ob", bufs=1)
            psB = pool(name="psB", bufs=3, space="PSUM")
            psPV = pool(name="psPV", bufs=4, space="PSUM")

            KT = res.tile([P, NT, S], BF16, tag="kt")
            V = res.tile([P, S // P, D], BF16, tag="v")
            QT = res.tile([P, NT, NQ], BF16, tag="qt")
            attnT = res.tile([P, NT, NQ], BF16, tag="at")
            maskst = res.tile([P, 16, QB], BF16, tag="msk")
            nc.gpsimd.dma_start(maskst[:], inp["masks"][:])
            ones_col = smp.tile([P, 1], F32R, tag="onescol")
            nc.sync.dma_start(ones_col[:], inp["ones_col"][:])
            ones_row = smp.tile([1, P], F32, tag="onesrow")
            nc.sync.dma_start(ones_row[:], inp["ones_row"][:])

            def rope_block(dst, src, cos_t, sin_t):
                # dst/src: [P, NT, w]; rows 0:NT/2 = even feats, NT/2: = odd
                h = NT // 2
                w = src.shape[-1]
                e, o = src[:, 0:h, :], src[:, h:NT, :]
                c, s = cos_t[:, :, :], sin_t[:, :, :]
                t1 = tmpp.tile([P, h, SB], BF16, tag="t1")
                nc.vector.tensor_mul(out=dst[:, 0:h, :], in0=e, in1=c)
                nc.vector.tensor_mul(out=t1[:, :, 0:w], in0=o, in1=s)
                nc.vector.tensor_tensor(dst[:, 0:h, :], dst[:, 0:h, :],
                                        t1[:, :, 0:w], mybir.AluOpType.subtract)
                t2 = tmpp.tile([P, h, SB], BF16, tag="t1")
                nc.vector.tensor_mul(out=dst[:, h:NT, :], in0=o, in1=c)
                nc.vector.tensor_mul(out=t2[:, :, 0:w], in0=e, in1=s)
                nc.vector.tensor_tensor(dst[:, h:NT, :], dst[:, h:NT, :],
                                        t2[:, :, 0:w], mybir.AluOpType.add)

            # ---------- Phase 1: Q^T projection + RoPE (2 halves) -----------
            WqRes = wres.tile([P, NT, D], BF16, tag="wqo")
            for t in range(NT):
                nc.sync.dma_start(WqRes[:, t, :], WqT_r[:, t, :])
            WkRes = wres.tile([P, NT, D], BF16, tag="wk")
            for t in range(NT):
                nc.sync.dma_start(WkRes[:, t, :], WkT_r[:, t, :])
            WvRes = wres.tile([P, NT, D], BF16, tag="wv")
            for t in range(NT):
                nc.sync.dma_start(WvRes[:, t, :], WvT_r[:, t, :])

            for qh in range(2):
                sl = slice(qh * SB, (qh + 1) * SB)
                xq = xbp.tile([P, NT, SB], BF16, tag="xb")
                for t in range(NT):
                    nc.sync.dma_start(xq[:, t, :], xTq_r[:, t, sl])
                cq = csp.tile([P, NT // 2, SB], BF16, tag="cs_c")
                nc.gpsimd.dma_start(cq[:], cosQ_r[:, :, sl])
                sq = csp.tile([P, NT // 2, SB], BF16, tag="cs_s")
                nc.gpsimd.dma_start(sq[:], sinQ_r[:, :, sl])
                qraw = rawp.tile([P, NT, SB], BF16, tag="raw")
                for t_out in range(NT):
                    ps = psB.tile([P, SB], F32, tag="psB")
                    for dt_ in range(NT):
                        nc.tensor.matmul(ps[:], WqRes[:, dt_, t_out * P:(t_out + 1) * P],
                                         xq[:, dt_, :], start=(dt_ == 0), stop=(dt_ == NT - 1))
                    nc.vector.tensor_copy(qraw[:, t_out, :], ps[:])
                rope_block(QT[:, :, sl], qraw, cq, sq)

            # Wo shares the WqRes slot; reload it now (Wq no longer needed).
            WoRes = wres.tile([P, NT, D], BF16, tag="wqo")
            for t in range(NT):
                nc.sync.dma_start(WoRes[:, t, :], WoT_r[:, t, :])

            # ---------- Interleaved: K/V projection block sb, then slot sb ----
            for sb in range(S // SB):
                sl = slice(sb * SB, (sb + 1) * SB)
                xb = xbp.tile([P, NT, SB], BF16, tag="xb")
                for t in range(NT):
                    nc.sync.dma_start(xb[:, t, :], xT_r[:, t, sl])
                # K^T block
                kraw = rawp.tile([P, NT, SB], BF16, tag="raw")
                for t_out in range(NT):
                    ps = psB.tile([P, SB], F32, tag="psB")
                    for dt_ in range(NT):
                        nc.tensor.matmul(ps[:], WkRes[:, dt_, t_out * P:(t_out + 1) * P],
                                         xb[:, dt_, :], start=(dt_ == 0), stop=(dt_ == NT - 1))
                    nc.vector.tensor_copy(kraw[:, t_out, :], ps[:])
                ck = csp.tile([P, NT // 2, SB], BF16, tag="cs_c")
                nc.gpsimd.dma_start(ck[:], cosK_r[:, :, sl])
                sk = csp.tile([P, NT // 2, SB], BF16, tag="cs_s")
                nc.gpsimd.dma_start(sk[:], sinK_r[:, :, sl])
                rope_block(KT[:, :, sl], kraw, ck, sk)
                # V rows for this block
                for sk_ in range(SB // P):
                    for dh in range(2):
                        ps = psB.tile([P, SB], F32, tag="psB")
                        for dt_ in range(NT):
                            nc.tensor.matmul(ps[:], xb[:, dt_, sk_ * P:(sk_ + 1) * P],
                                             WvRes[:, dt_, dh * SB:(dh + 1) * SB],
                                             start=(dt_ == 0), stop=(dt_ == NT - 1))
                        nc.scalar.copy(V[:, sb * (SB // P) + sk_, dh * SB:(dh + 1) * SB], ps[:])

                # ---------- wave B of the previous slot (PE overlap) --------
                if sb > 0:
                    pc, pqsl, pPT, pbc = prev
                    pvB = [psPV.tile([P, SB], F32, tag="pv", name=f"pvB{sb}_{j}")
                           for j in range(4)]
                    for v in range(pc):
                        for j in range(4):
                            nc.tensor.matmul(pvB[j][:, 0:QB],
                                             V[:, v, (4 + j) * P:(5 + j) * P],
                                             pPT[:, v, :], start=(v == 0),
                                             stop=(v == pc - 1))
                    for j in range(4):
                        nc.vector.tensor_mul(out=attnT[:, 4 + j, pqsl],
                                             in0=pvB[j][:, 0:QB], in1=pbc[:])

                # ---------- attention slot sb (wave A in-loop) ----------
                s = sb
                c = NVIS[s]
                qsl = slice(s * QB, (s + 1) * QB)
                sumacc = sap.tile([P, QB], F32R, tag="sa")
                PT = ptp.tile([P, 16, QB], BF16, tag="pts")
                pvA = [psPV.tile([P, SB], F32, tag="pv", name=f"pvA{s}_{j}")
                       for j in range(4)]
                for v in range(c):
                    ps = psB.tile([P, SB], F32, tag="psB")
                    for dt_ in range(NT):
                        nc.tensor.matmul(ps[:, 0:QB], KT[:, dt_, v * P:(v + 1) * P],
                                         QT[:, dt_, qsl], start=(dt_ == 0), stop=(dt_ == NT - 1))
                    nc.scalar.activation(PT[:, v, :], ps[:, 0:QB],
                                         mybir.ActivationFunctionType.Exp, scale=SCALE)
                    if v >= c - 4:
                        nc.vector.tensor_mul(out=PT[:, v, :], in0=PT[:, v, :],
                                             in1=maskst[:, 4 * s + (v - (c - 4)), :])
                    if v == 0:
                        nc.vector.tensor_copy(sumacc[:], PT[:, v, :])
                    else:
                        nc.vector.tensor_tensor(sumacc[:], sumacc[:], PT[:, v, :],
                                                mybir.AluOpType.add)
                    if v > 0:
                        for j in range(4):
                            nc.tensor.matmul(pvA[j][:, 0:QB],
                                             V[:, v - 1, j * P:(j + 1) * P], PT[:, v - 1, :],
                                             start=(v - 1 == 0), stop=False)
                for j in range(4):
                    nc.tensor.matmul(pvA[j][:, 0:QB],
                                     V[:, c - 1, j * P:(j + 1) * P], PT[:, c - 1, :],
                                     start=(c == 1), stop=True)
                # normalize: 1/rowsum broadcast via PE, scale wave-A chunks
                sums_ps = psB.tile([P, SB], F32, tag="psB")
                nc.tensor.matmul(sums_ps[0:1, 0:QB], ones_col[:], sumacc[:],
                                 start=True, stop=True)
                recip = smp.tile([1, QB], F32, tag="recip")
                nc.vector.reciprocal(recip[:], sums_ps[0:1, 0:QB])
                bc_ps = psB.tile([P, SB], F32, tag="psB")
                nc.tensor.matmul(bc_ps[:, 0:QB], ones_row[:], recip[:],
                                 start=True, stop=True)
                bc = bcp.tile([P, QB], F32, tag="bc")
                nc.scalar.copy(bc[:], bc_ps[:, 0:QB])
                for j in range(4):
                    nc.vector.tensor_mul(out=attnT[:, j, qsl],
                                         in0=pvA[j][:, 0:QB], in1=bc[:])
                prev = (c, qsl, PT, bc)

            # ---------- wave B of the last slot ----------
            pc, pqsl, pPT, pbc = prev
            pvB = [psPV.tile([P, SB], F32, tag="pv", name=f"pvBf_{j}")
                   for j in range(4)]
            for v in range(pc):
                for j in range(4):
                    nc.tensor.matmul(pvB[j][:, 0:QB],
                                     V[:, v, (4 + j) * P:(5 + j) * P],
                                     pPT[:, v, :], start=(v == 0), stop=(v == pc - 1))
            for j in range(4):
                nc.vector.tensor_mul(out=attnT[:, 4 + j, pqsl],
                                     in0=pvB[j][:, 0:QB], in1=pbc[:])

            # ---------- output projection ----------
            for qh in range(4):
                sl = slice(qh * QB, (qh + 1) * QB)
                ob = obp.tile([P, NT, QB], BF16, tag="ob")
                for oc in range(NT):
                    ps = psB.tile([P, SB], F32, tag="psB")
                    for dt_ in range(NT):
                        nc.tensor.matmul(ps[:, 0:QB], WoRes[:, dt_, oc * P:(oc + 1) * P],
                                         attnT[:, dt_, sl], start=(dt_ == 0), stop=(dt_ == NT - 1))
                    nc.scalar.copy(ob[:, oc, :], ps[:, 0:QB])
                nc.sync.dma_start(outT[:, :, sl], ob[:])

    nc.finalize()
    return nc


def _host_inputs(x, Wq, Wk, Wv, Wo, token_positions):
    import ml_dtypes
    bf = ml_dtypes.bfloat16
    perm = np.concatenate([np.arange(0, D, 2), np.arange(1, D, 2)])
    WqTp = np.ascontiguousarray(Wq[perm].T).astype(bf)
    WkTp = np.ascontiguousarray(Wk[perm].T).astype(bf)
    WvT = np.ascontiguousarray(Wv.T).astype(bf)
    WoT = np.ascontiguousarray(Wo.T).astype(bf)
    inv_freq = (1.0 / (np.float32(THETA) **
                       (np.arange(0, D, 2, dtype=np.float32) / np.float32(D))))
    ones_col = np.ones((P, 1), np.float32)
    ones_row = np.ones((1, P), np.float32)

    in_maps, metas = [], []
    for b in range(B):
        xT = np.ascontiguousarray(x[b].T).astype(bf)           # [D, S]
        pos = token_positions[b].astype(np.float32)
        ang = (pos[None, :] * inv_freq[:, None]).astype(np.float32)  # [D/2, S]
        cosF = np.cos(ang)
        sinF = np.sin(ang)
        for h in range(2):
            blocks = BLOCKS[h]
            qcols = np.concatenate([np.arange(QB * bs, QB * (bs + 1))
                                    for bs in blocks])
            xTq = np.ascontiguousarray(xT[:, qcols])
            cosQ = np.ascontiguousarray(cosF[:, qcols]).astype(bf)
            sinQ = np.ascontiguousarray(sinF[:, qcols]).astype(bf)
            m = np.zeros((P, 16, QB), dtype=np.float32)
            for s, bs in enumerate(blocks):
                c = NVIS[s]
                q0 = QB * bs
                q_glob = q0 + np.arange(QB)
                for j in range(4):
                    v = c - 4 + j
                    k_glob = 128 * v + np.arange(P)
                    m[:, 4 * s + j, :] = (q_glob[None, :] >= k_glob[:, None])
            in_maps.append({
                "ones_col": ones_col, "ones_row": ones_row,
                "xT": xT, "xTq": xTq,
                "WqT": WqTp, "WkT": WkTp, "WvT": WvT, "WoT": WoT,
                "cosK": cosF.astype(bf), "sinK": sinF.astype(bf),
                "cosQ": cosQ, "sinQ": sinQ,
                "masks": m.astype(bf),
            })
            metas.append((b, qcols))
    return in_maps, metas


_NC_CACHE = {}


def kernel(x, Wq, Wk, Wv, Wo, token_positions):
    x = np.asarray(x); token_positions = np.asarray(token_positions)
    if "nc" not in _NC_CACHE:
        _NC_CACHE["nc"] = _build_program()
    nc = _NC_CACHE["nc"]
    in_maps, metas = _host_inputs(np.asarray(x), np.asarray(Wq), np.asarray(Wk),
                                  np.asarray(Wv), np.asarray(Wo), token_positions)
    res = run_bass_kernel_spmd(nc, in_maps, core_ids=list(range(8)))
    out = np.empty((B, S, D), dtype=np.float32)
    for (b, qcols), r in zip(metas, res.results):
        oT = np.asarray(r["outT"]).astype(np.float32)   # [P, NT, NQ]
        o = np.transpose(oT, (2, 1, 0)).reshape(NQ, D)
        out[b, qcols, :] = o
    return out


# revision 9
# speedup vs baseline: 1.3185x; 1.0580x over previous
"""Causal no-head self-attention with RoPE on 8 Trainium2 NeuronCores.

Sharding: 8 cores = 4 batches x 2 query-sets of four 256-query blocks.
Per-core slots s=0..3 run 4(s+1) key-tile visits (128 keys each); block
assignment (h=0: blocks {1,3,4,7}, h=1: {0,2,5,6}) makes the same
(4,8,12,16) visit structure causally sufficient on every core, so all
cores run ONE identical Bass program; per-core differences (which
queries, causal masks, RoPE angles) are carried in the input data.

Everything is bf16 (PE full rate, half DMA/SBUF of fp32) with fp32 PSUM
accumulation. K^T and V live entirely in SBUF (no DRAM spill). Device
layouts are transposed, channel-on-partition: QT/KT [d_k, q|s] with d_k
de-interleaved (even feats rows 0:512, odd 512:1024) so RoPE is a
contiguous-partition-block rotation; the same permutation is applied to
Wq/Wk output columns on host (scores are invariant). V natural [s, d_v].
"""

import numpy as np
import sys

for _p in ("/opt/trn_rl_repo",):
    if _p not in sys.path:
        sys.path.insert(0, _p)

import concourse.bass as bass
import concourse.bacc as bacc
import concourse.mybir as mybir
from concourse.tile import TileContext
from concourse.bass_utils import run_bass_kernel_spmd

B, S, D = 4, 2048, 1024
THETA = 10000.0
P = 128
NT = D // P          # 8 partition-tiles over the 1024 dim
SB = 512             # seq block width for K/V projection (4 blocks)
QB = 256             # query slot width (4 slots per core)
NQ = 1024            # queries per core
F32 = mybir.dt.float32
F32R = mybir.dt.float32r
BF16 = mybir.dt.bfloat16
SCALE = 1.0 / 32.0   # 1/sqrt(D)
BLOCKS = [[1, 3, 4, 7], [0, 2, 5, 6]]   # 256-query blocks per core half
NVIS = [4, 8, 12, 16]                    # kt visits per slot (same all cores)


def _build_program():
    nc = bacc.Bacc("TRN2", num_swdge_queues=4)
    inp = {}
    def din(name, shape, dt):
        inp[name] = nc.dram_tensor(name, shape, dt, kind="ExternalInput")
    din("xT", [D, S], BF16)
    din("xTq", [D, NQ], BF16)
    din("WqT", [D, D], BF16)
    din("WkT", [D, D], BF16)
    din("WvT", [D, D], BF16)
    din("WoT", [D, D], BF16)
    din("cosK", [D // 2, S], BF16)
    din("sinK", [D // 2, S], BF16)
    din("cosQ", [D // 2, NQ], BF16)
    din("sinQ", [D // 2, NQ], BF16)
    din("masks", [P, 16, QB], BF16)
    din("ones_col", [P, 1], F32R)
    din("ones_row", [1, P], F32R)
    outT = nc.dram_tensor("outT", [P, NT, NQ], BF16, kind="ExternalOutput")

    xT_r = inp["xT"].rearrange("(t p) s -> p t s", p=P)
    xTq_r = inp["xTq"].rearrange("(t p) s -> p t s", p=P)
    WqT_r = inp["WqT"].rearrange("(t p) o -> p t o", p=P)
    WkT_r = inp["WkT"].rearrange("(t p) o -> p t o", p=P)
    WvT_r = inp["WvT"].rearrange("(t p) o -> p t o", p=P)
    WoT_r = inp["WoT"].rearrange("(t p) o -> p t o", p=P)
    cosK_r = inp["cosK"].rearrange("(t p) s -> p t s", p=P)
    sinK_r = inp["sinK"].rearrange("(t p) s -> p t s", p=P)
    cosQ_r = inp["cosQ"].rearrange("(t p) s -> p t s", p=P)
    sinQ_r = inp["sinQ"].rearrange("(t p) s -> p t s", p=P)

    from contextlib import ExitStack
    with TileContext(nc) as tc:
        with ExitStack() as ctx:
            pool = lambda *a, **kw: ctx.enter_context(tc.tile_pool(*a, **kw))
            res = pool(name="res", bufs=1)          # big residents
            wres = pool(name="wres", bufs=1)        # weights
            smp = pool(name="small", bufs=1)
            xbp = pool(name="xb", bufs=2)
            csp = pool(name="cs", bufs=2)
            rawp = pool(name="raw", bufs=2)
            tmpp = pool(name="tmp", bufs=1)
            ptp = pool(name="pt", bufs=1)
            sap = pool(name="sa", bufs=1)
            bcp = pool(name="bc", bufs=2)
            obp = pool(name="ob", bufs=1)
            psB = pool(name="psB", bufs=3, space="PSUM")
            psPV = pool(name="psPV", bufs=4, space="PSUM")

            KT = res.tile([P, NT, S], BF16, tag="kt")
            V = res.tile([P, S // P, D], BF16, tag="v")
            # QT doubles as attnT: slot s's normalized PV overwrites QT's
            # columns after the slot's scores are done reading them.
            QT = res.tile([P, NT, NQ], BF16, tag="qt")
            attnT = QT
            maskst = res.tile([P, 16, QB], BF16, tag="msk")
            nc.gpsimd.dma_start(maskst[:], inp["masks"][:])
            ones_col = smp.tile([P, 1], F32R, tag="onescol")
            nc.sync.dma_start(ones_col[:], inp["ones_col"][:])
            ones_row = smp.tile([1, P], F32R, tag="onesrow")
            nc.sync.dma_start(ones_row[:], inp["ones_row"][:])

            def rope_block(dst, src, cos_t, sin_t):
                # dst/src: [P, NT, w]; rows 0:NT/2 = even feats, NT/2: = odd
                h = NT // 2
                w = src.shape[-1]
                e, o = src[:, 0:h, :], src[:, h:NT, :]
                c, s = cos_t[:, :, :], sin_t[:, :, :]
                t1 = tmpp.tile([P, h, SB], BF16, tag="t1")
                nc.vector.tensor_mul(out=dst[:, 0:h, :], in0=e, in1=c)
                nc.vector.tensor_mul(out=t1[:, :, 0:w], in0=o, in1=s)
                nc.vector.tensor_tensor(dst[:, 0:h, :], dst[:, 0:h, :],
                                        t1[:, :, 0:w], mybir.AluOpType.subtract)
                t2 = tmpp.tile([P, h, SB], BF16, tag="t1")
                nc.vector.tensor_mul(out=dst[:, h:NT, :], in0=o, in1=c)
                nc.vector.tensor_mul(out=t2[:, :, 0:w], in0=e, in1=s)
                nc.vector.tensor_tensor(dst[:, h:NT, :], dst[:, h:NT, :],
                                        t2[:, :, 0:w], mybir.AluOpType.add)

            # ---------- Phase 1: Q^T projection + RoPE (2 halves) -----------
            # DMA issue order matters: x/cos/sin first so the first matmul
            # isn't gated behind 6MB of weight loads.
            xqs, cqs, sqs = [], [], []
            for qh in range(2):
                sl = slice(qh * SB, (qh + 1) * SB)
                xq = xbp.tile([P, NT, SB], BF16, tag="xb")
                for t in range(NT):
                    nc.sync.dma_start(xq[:, t, :], xTq_r[:, t, sl])
                cq = csp.tile([P, NT // 2, SB], BF16, tag="cs_c")
                nc.gpsimd.dma_start(cq[:], cosQ_r[:, :, sl])
                sq = csp.tile([P, NT // 2, SB], BF16, tag="cs_s")
                nc.gpsimd.dma_start(sq[:], sinQ_r[:, :, sl])
                xqs.append(xq); cqs.append(cq); sqs.append(sq)
            WqRes = wres.tile([P, NT, D], BF16, tag="wqo")
            for t in range(NT):
                nc.sync.dma_start(WqRes[:, t, :], WqT_r[:, t, :])
            WkRes = wres.tile([P, NT, D], BF16, tag="wk")
            WvRes = wres.tile([P, NT, D], BF16, tag="wv")

            for qh in range(2):
                sl = slice(qh * SB, (qh + 1) * SB)
                xq, cq, sq = xqs[qh], cqs[qh], sqs[qh]
                qraw = rawp.tile([P, NT, SB], BF16, tag="raw")
                for t_out in range(NT):
                    ps = psB.tile([P, SB], F32, tag="psB")
                    for dt_ in range(NT):
                        nc.tensor.matmul(ps[:], WqRes[:, dt_, t_out * P:(t_out + 1) * P],
                                         xq[:, dt_, :], start=(dt_ == 0), stop=(dt_ == NT - 1))
                    nc.vector.tensor_copy(qraw[:, t_out, :], ps[:])
                rope_block(QT[:, :, sl], qraw, cq, sq)

            # K/V weights stream in behind the Q-phase compute (gpsimd issue
            # keeps the sync queue free for x loads).
            for t in range(NT):
                nc.gpsimd.dma_start(WkRes[:, t, :], WkT_r[:, t, :])
            for t in range(NT):
                nc.gpsimd.dma_start(WvRes[:, t, :], WvT_r[:, t, :])
            # Wo shares the WqRes slot; reload it now (Wq no longer needed).
            WoRes = wres.tile([P, NT, D], BF16, tag="wqo")
            for t in range(NT):
                nc.sync.dma_start(WoRes[:, t, :], WoT_r[:, t, :])

            # ---------- Interleaved: K/V projection block sb, then slot sb ----
            for sb in range(S // SB):
                sl = slice(sb * SB, (sb + 1) * SB)
                xb = xbp.tile([P, NT, SB], BF16, tag="xb")
                for t in range(NT):
                    nc.sync.dma_start(xb[:, t, :], xT_r[:, t, sl])
                # K^T block
                kraw = rawp.tile([P, NT, SB], BF16, tag="raw")
                for t_out in range(NT):
                    ps = psB.tile([P, SB], F32, tag="psB")
                    for dt_ in range(NT):
                        nc.tensor.matmul(ps[:], WkRes[:, dt_, t_out * P:(t_out + 1) * P],
                                         xb[:, dt_, :], start=(dt_ == 0), stop=(dt_ == NT - 1))
                    nc.vector.tensor_copy(kraw[:, t_out, :], ps[:])
                ck = csp.tile([P, NT // 2, SB], BF16, tag="cs_c")
                nc.gpsimd.dma_start(ck[:], cosK_r[:, :, sl])
                sk = csp.tile([P, NT // 2, SB], BF16, tag="cs_s")
                nc.gpsimd.dma_start(sk[:], sinK_r[:, :, sl])
                rope_block(KT[:, :, sl], kraw, ck, sk)
                # V rows for this block
                for sk_ in range(SB // P):
                    for dh in range(2):
                        ps = psB.tile([P, SB], F32, tag="psB")
                        for dt_ in range(NT):
                            nc.tensor.matmul(ps[:], xb[:, dt_, sk_ * P:(sk_ + 1) * P],
                                             WvRes[:, dt_, dh * SB:(dh + 1) * SB],
                                             start=(dt_ == 0), stop=(dt_ == NT - 1))
                        nc.scalar.copy(V[:, sb * (SB // P) + sk_, dh * SB:(dh + 1) * SB], ps[:])

                # ---------- wave B of the previous slot (PE overlap) --------
                if sb > 0:
                    pc, pqsl, pPT, pbc = prev
                    pvB = [psPV.tile([P, SB], F32, tag="pv", name=f"pvB{sb}_{j}")
                           for j in range(4)]
                    for v in range(pc):
                        for j in range(4):
                            nc.tensor.matmul(pvB[j][:, 0:QB],
                                             V[:, v, (4 + j) * P:(5 + j) * P],
                                             pPT[:, v, :], start=(v == 0),
                                             stop=(v == pc - 1))
                    for j in range(4):
                        nc.vector.tensor_mul(out=attnT[:, 4 + j, pqsl],
                                             in0=pvB[j][:, 0:QB], in1=pbc[:])

                # ---------- attention slot sb (wave A in-loop) ----------
                s = sb
                c = NVIS[s]
                qsl = slice(s * QB, (s + 1) * QB)
                sumacc = sap.tile([P, QB], F32R, tag="sa")
                PT = ptp.tile([P, 16, QB], BF16, tag="pts")
                pvA = [psPV.tile([P, SB], F32, tag="pv", name=f"pvA{s}_{j}")
                       for j in range(4)]
                for v in range(c):
                    ps = psB.tile([P, SB], F32, tag="psB")
                    for dt_ in range(NT):
                        nc.tensor.matmul(ps[:, 0:QB], KT[:, dt_, v * P:(v + 1) * P],
                                         QT[:, dt_, qsl], start=(dt_ == 0), stop=(dt_ == NT - 1))
                    nc.scalar.activation(PT[:, v, :], ps[:, 0:QB],
                                         mybir.ActivationFunctionType.Exp, scale=SCALE)
                    if v >= c - 4:
                        nc.vector.tensor_mul(out=PT[:, v, :], in0=PT[:, v, :],
                                             in1=maskst[:, 4 * s + (v - (c - 4)), :])
                    if v == 0:
                        nc.vector.tensor_copy(sumacc[:], PT[:, v, :])
                    else:
                        nc.vector.tensor_tensor(sumacc[:], sumacc[:], PT[:, v, :],
                                                mybir.AluOpType.add)
                    if v > 0:
                        for j in range(4):
                            nc.tensor.matmul(pvA[j][:, 0:QB],
                                             V[:, v - 1, j * P:(j + 1) * P], PT[:, v - 1, :],
                                             start=(v - 1 == 0), stop=False)
                for j in range(4):
                    nc.tensor.matmul(pvA[j][:, 0:QB],
                                     V[:, c - 1, j * P:(j + 1) * P], PT[:, c - 1, :],
                                     start=(c == 1), stop=True)
                # normalize: 1/rowsum broadcast via PE, scale wave-A chunks
                sums_ps = psB.tile([P, SB], F32, tag="psB")
                nc.tensor.matmul(sums_ps[0:1, 0:QB], ones_col[:], sumacc[:],
                                 start=True, stop=True)
                sumrow = smp.tile([1, QB], F32R, tag="sumrow")
                nc.scalar.copy(sumrow[:], sums_ps[0:1, 0:QB])
                bc_ps = psB.tile([P, SB], F32, tag="psB")
                nc.tensor.matmul(bc_ps[:, 0:QB], ones_row[:], sumrow[:],
                                 start=True, stop=True)
                bc_sums = bcp.tile([P, QB], F32R, tag="bcs")
                nc.scalar.copy(bc_sums[:], bc_ps[:, 0:QB])
                bc = bcp.tile([P, QB], F32, tag="bc")
                nc.vector.reciprocal(bc[:], bc_sums[:])
                for j in range(4):
                    nc.vector.tensor_mul(out=attnT[:, j, qsl],
                                         in0=pvA[j][:, 0:QB], in1=bc[:])
                prev = (c, qsl, PT, bc)

            # ---------- wave B of the last slot ----------
            pc, pqsl, pPT, pbc = prev
            pvB = [psPV.tile([P, SB], F32, tag="pv", name=f"pvBf_{j}")
                   for j in range(4)]
            for v in range(pc):
                for j in range(4):
                    nc.tensor.matmul(pvB[j][:, 0:QB],
                                     V[:, v, (4 + j) * P:(5 + j) * P],
                                     pPT[:, v, :], start=(v == 0), stop=(v == pc - 1))
            for j in range(4):
                nc.vector.tensor_mul(out=attnT[:, 4 + j, pqsl],
                                     in0=pvB[j][:, 0:QB], in1=pbc[:])

            # ---------- output projection ----------
            for qh in range(4):
                sl = slice(qh * QB, (qh + 1) * QB)
                ob = obp.tile([P, NT, QB], BF16, tag="ob")
                for oc in range(NT):
                    ps = psB.tile([P, SB], F32, tag="psB")
                    for dt_ in range(NT):
                        nc.tensor.matmul(ps[:, 0:QB], WoRes[:, dt_, oc * P:(oc + 1) * P],
                                         attnT[:, dt_, sl], start=(dt_ == 0), stop=(dt_ == NT - 1))
                    nc.scalar.copy(ob[:, oc, :], ps[:, 0:QB])
                    nc.sync.dma_start(outT[:, oc, sl], ob[:, oc, :])

    nc.finalize()
    return nc


def _host_inputs(x, Wq, Wk, Wv, Wo, token_positions):
    import ml_dtypes
    bf = ml_dtypes.bfloat16
    perm = np.concatenate([np.arange(0, D, 2), np.arange(1, D, 2)])
    WqTp = np.ascontiguousarray(Wq[perm].T).astype(bf)
    WkTp = np.ascontiguousarray(Wk[perm].T).astype(bf)
    WvT = np.ascontiguousarray(Wv.T).astype(bf)
    WoT = np.ascontiguousarray(Wo.T).astype(bf)
    inv_freq = (1.0 / (np.float32(THETA) **
                       (np.arange(0, D, 2, dtype=np.float32) / np.float32(D))))
    ones_col = np.ones((P, 1), np.float32)
    ones_row = np.ones((1, P), np.float32)

    in_maps, metas = [], []
    for b in range(B):
        xT = np.ascontiguousarray(x[b].T).astype(bf)           # [D, S]
        pos = token_positions[b].astype(np.float32)
        ang = (pos[None, :] * inv_freq[:, None]).astype(np.float32)  # [D/2, S]
        cosF = np.cos(ang)
        sinF = np.sin(ang)
        for h in range(2):
            blocks = BLOCKS[h]
            qcols = np.concatenate([np.arange(QB * bs, QB * (bs + 1))
                                    for bs in blocks])
            xTq = np.ascontiguousarray(xT[:, qcols])
            cosQ = np.ascontiguousarray(cosF[:, qcols]).astype(bf)
            sinQ = np.ascontiguousarray(sinF[:, qcols]).astype(bf)
            m = np.zeros((P, 16, QB), dtype=np.float32)
            for s, bs in enumerate(blocks):
                c = NVIS[s]
                q0 = QB * bs
                q_glob = q0 + np.arange(QB)
                for j in range(4):
                    v = c - 4 + j
                    k_glob = 128 * v + np.arange(P)
                    m[:, 4 * s + j, :] = (q_glob[None, :] >= k_glob[:, None])
            in_maps.append({
                "ones_col": ones_col, "ones_row": ones_row,
                "xT": xT, "xTq": xTq,
                "WqT": WqTp, "WkT": WkTp, "WvT": WvT, "WoT": WoT,
                "cosK": cosF.astype(bf), "sinK": sinF.astype(bf),
                "cosQ": cosQ, "sinQ": sinQ,
                "masks": m.astype(bf),
            })
            metas.append((b, qcols))
    return in_maps, metas


_NC_CACHE = {}


def kernel(x, Wq, Wk, Wv, Wo, token_positions):
    x = np.asarray(x); token_positions = np.asarray(token_positions)
    if "nc" not in _NC_CACHE:
        _NC_CACHE["nc"] = _build_program()
    nc = _NC_CACHE["nc"]
    in_maps, metas = _host_inputs(np.asarray(x), np.asarray(Wq), np.asarray(Wk),
                                  np.asarray(Wv), np.asarray(Wo), token_positions)
    res = run_bass_kernel_spmd(nc, in_maps, core_ids=list(range(8)))
    out = np.empty((B, S, D), dtype=np.float32)
    for (b, qcols), r in zip(metas, res.results):
        oT = np.asarray(r["outT"]).astype(np.float32)   # [P, NT, NQ]
        o = np.transpose(oT, (2, 1, 0)).reshape(NQ, D)
        out[b, qcols, :] = o
    return out


# revision 10
# speedup vs baseline: 1.3326x; 1.0107x over previous
"""Causal no-head self-attention with RoPE on 8 Trainium2 NeuronCores.

Sharding: 8 cores = 4 batches x 2 query-sets of four 256-query blocks.
Per-core slots s=0..3 run 4(s+1) key-tile visits (128 keys each); block
assignment (h=0: blocks {1,3,4,7}, h=1: {0,2,5,6}) makes the same
(4,8,12,16) visit structure causally sufficient on every core, so all
cores run ONE identical Bass program; per-core differences (which
queries, causal masks, RoPE angles) are carried in the input data.

Everything is bf16 (PE full rate, half DMA/SBUF of fp32) with fp32 PSUM
accumulation. K^T and V live entirely in SBUF (no DRAM spill). Device
layouts are transposed, channel-on-partition: QT/KT [d_k, q|s] with d_k
de-interleaved (even feats rows 0:512, odd 512:1024) so RoPE is a
contiguous-partition-block rotation; the same permutation is applied to
Wq/Wk output columns on host (scores are invariant). V natural [s, d_v].
"""

import numpy as np
import sys

for _p in ("/opt/trn_rl_repo",):
    if _p not in sys.path:
        sys.path.insert(0, _p)

import concourse.bass as bass
import concourse.bacc as bacc
import concourse.mybir as mybir
from concourse.tile import TileContext
from concourse.bass_utils import run_bass_kernel_spmd

B, S, D = 4, 2048, 1024
THETA = 10000.0
P = 128
NT = D // P          # 8 partition-tiles over the 1024 dim
SB = 512             # seq block width for K/V projection (4 blocks)
QB = 256             # query slot width (4 slots per core)
NQ = 1024            # queries per core
F32 = mybir.dt.float32
F32R = mybir.dt.float32r
BF16 = mybir.dt.bfloat16
SCALE = 1.0 / 32.0   # 1/sqrt(D)
BLOCKS = [[1, 3, 4, 7], [0, 2, 5, 6]]   # 256-query blocks per core half
NVIS = [4, 8, 12, 16]                    # kt visits per slot (same all cores)


def _build_program():
    nc = bacc.Bacc("TRN2", num_swdge_queues=4)
    inp = {}
    def din(name, shape, dt):
        inp[name] = nc.dram_tensor(name, shape, dt, kind="ExternalInput")
    din("xT", [D, S], BF16)
    din("xTq", [D, NQ], BF16)
    din("WqT", [D, D], BF16)
    din("WkT", [D, D], BF16)
    din("WvT", [D, D], BF16)
    din("WoT", [D, D], BF16)
    din("cosK", [D // 2, S], BF16)
    din("sinK", [D // 2, S], BF16)
    din("cosQ", [D // 2, NQ], BF16)
    din("sinQ", [D // 2, NQ], BF16)
    din("masks", [P, 16, QB], BF16)
    din("ones_col", [P, 1], F32R)
    din("ones_row", [1, P], F32R)
    outT = nc.dram_tensor("outT", [P, NT, NQ], BF16, kind="ExternalOutput")

    xT_r = inp["xT"].rearrange("(t p) s -> p t s", p=P)
    xTq_r = inp["xTq"].rearrange("(t p) s -> p t s", p=P)
    WqT_r = inp["WqT"].rearrange("(t p) o -> p t o", p=P)
    WkT_r = inp["WkT"].rearrange("(t p) o -> p t o", p=P)
    WvT_r = inp["WvT"].rearrange("(t p) o -> p t o", p=P)
    WoT_r = inp["WoT"].rearrange("(t p) o -> p t o", p=P)
    cosK_r = inp["cosK"].rearrange("(t p) s -> p t s", p=P)
    sinK_r = inp["sinK"].rearrange("(t p) s -> p t s", p=P)
    cosQ_r = inp["cosQ"].rearrange("(t p) s -> p t s", p=P)
    sinQ_r = inp["sinQ"].rearrange("(t p) s -> p t s", p=P)

    from contextlib import ExitStack
    with TileContext(nc) as tc:
        with ExitStack() as ctx:
            pool = lambda *a, **kw: ctx.enter_context(tc.tile_pool(*a, **kw))
            res = pool(name="res", bufs=1)          # big residents
            wres = pool(name="wres", bufs=1)        # weights
            smp = pool(name="small", bufs=1)
            xbp = pool(name="xb", bufs=2)
            csp = pool(name="cs", bufs=2)
            rawp = pool(name="raw", bufs=2)
            tmpp = pool(name="tmp", bufs=1)
            ptp = pool(name="pt", bufs=1)
            sap = pool(name="sa", bufs=1)
            bcp = pool(name="bc", bufs=2)
            obp = pool(name="ob", bufs=1)
            psB = pool(name="psB", bufs=3, space="PSUM")
            psPV = pool(name="psPV", bufs=4, space="PSUM")

            KT = res.tile([P, NT, S], BF16, tag="kt")
            V = res.tile([P, S // P, D], BF16, tag="v")
            # QT doubles as attnT: slot s's normalized PV overwrites QT's
            # columns after the slot's scores are done reading them.
            QT = res.tile([P, NT, NQ], BF16, tag="qt")
            attnT = QT
            maskst = res.tile([P, 16, QB], BF16, tag="msk")
            ones_col = smp.tile([P, 1], F32R, tag="onescol")
            nc.sync.dma_start(ones_col[:], inp["ones_col"][:])
            ones_row = smp.tile([1, P], F32R, tag="onesrow")
            nc.sync.dma_start(ones_row[:], inp["ones_row"][:])

            def rope_block(dst, src, cos_t, sin_t):
                # dst/src: [P, NT, w]; rows 0:NT/2 = even feats, NT/2: = odd
                h = NT // 2
                w = src.shape[-1]
                e, o = src[:, 0:h, :], src[:, h:NT, :]
                c, s = cos_t[:, :, :], sin_t[:, :, :]
                t1 = tmpp.tile([P, h, SB], BF16, tag="t1")
                nc.vector.tensor_mul(out=dst[:, 0:h, :], in0=e, in1=c)
                nc.vector.tensor_mul(out=t1[:, :, 0:w], in0=o, in1=s)
                nc.vector.tensor_tensor(dst[:, 0:h, :], dst[:, 0:h, :],
                                        t1[:, :, 0:w], mybir.AluOpType.subtract)
                t2 = tmpp.tile([P, h, SB], BF16, tag="t1")
                nc.vector.tensor_mul(out=dst[:, h:NT, :], in0=o, in1=c)
                nc.vector.tensor_mul(out=t2[:, :, 0:w], in0=e, in1=s)
                nc.vector.tensor_tensor(dst[:, h:NT, :], dst[:, h:NT, :],
                                        t2[:, :, 0:w], mybir.AluOpType.add)

            # ---------- Phase 1: Q^T projection + RoPE (2 halves) -----------
            # DMA issue order matters: x/cos/sin first so the first matmul
            # isn't gated behind 6MB of weight loads.
            xqs, cqs, sqs = [], [], []
            for qh in range(2):
                sl = slice(qh * SB, (qh + 1) * SB)
                xq = xbp.tile([P, NT, SB], BF16, tag="xb")
                for t in range(NT):
                    nc.sync.dma_start(xq[:, t, :], xTq_r[:, t, sl])
                cq = csp.tile([P, NT // 2, SB], BF16, tag="cs_c")
                nc.gpsimd.dma_start(cq[:], cosQ_r[:, :, sl])
                sq = csp.tile([P, NT // 2, SB], BF16, tag="cs_s")
                nc.gpsimd.dma_start(sq[:], sinQ_r[:, :, sl])
                xqs.append(xq); cqs.append(cq); sqs.append(sq)
                if qh == 0:
                    # Wq right behind the first x half, split small so all 16
                    # DMA queues pull it in parallel.
                    WqRes = wres.tile([P, NT, D], BF16, tag="wqo")
                    for t in range(NT):
                        for hh in range(2):
                            w_sl = slice(hh * SB, (hh + 1) * SB)
                            nc.sync.dma_start(WqRes[:, t, w_sl], WqT_r[:, t, w_sl])
            nc.gpsimd.dma_start(maskst[:], inp["masks"][:])
            WkRes = wres.tile([P, NT, D], BF16, tag="wk")
            for t in range(NT):
                for hh in range(2):
                    w_sl = slice(hh * SB, (hh + 1) * SB)
                    nc.sync.dma_start(WkRes[:, t, w_sl], WkT_r[:, t, w_sl])
            WvRes = wres.tile([P, NT, D], BF16, tag="wv")
            for t in range(NT):
                for hh in range(2):
                    w_sl = slice(hh * SB, (hh + 1) * SB)
                    nc.sync.dma_start(WvRes[:, t, w_sl], WvT_r[:, t, w_sl])

            for qh in range(2):
                sl = slice(qh * SB, (qh + 1) * SB)
                xq, cq, sq = xqs[qh], cqs[qh], sqs[qh]
                qraw = rawp.tile([P, NT, SB], BF16, tag="raw")
                for t_out in range(NT):
                    ps = psB.tile([P, SB], F32, tag="psB")
                    for dt_ in range(NT):
                        nc.tensor.matmul(ps[:], WqRes[:, dt_, t_out * P:(t_out + 1) * P],
                                         xq[:, dt_, :], start=(dt_ == 0), stop=(dt_ == NT - 1))
                    nc.vector.tensor_copy(qraw[:, t_out, :], ps[:])
                rope_block(QT[:, :, sl], qraw, cq, sq)

            # Wo shares the WqRes slot; allocated now, DMA'd during block 1
            # so it doesn't compete with the Wk/Wv/x loads the early blocks
            # are waiting on.
            WoRes = wres.tile([P, NT, D], BF16, tag="wqo")

            # ---------- Interleaved: K/V projection block sb, then slot sb ----
            for sb in range(S // SB):
                sl = slice(sb * SB, (sb + 1) * SB)
                xb = xbp.tile([P, NT, SB], BF16, tag="xb")
                for t in range(NT):
                    nc.sync.dma_start(xb[:, t, :], xT_r[:, t, sl])
                if sb == 1:
                    for t in range(NT):
                        nc.sync.dma_start(WoRes[:, t, :], WoT_r[:, t, :])
                # K^T block
                kraw = rawp.tile([P, NT, SB], BF16, tag="raw")
                for t_out in range(NT):
                    ps = psB.tile([P, SB], F32, tag="psB")
                    for dt_ in range(NT):
                        nc.tensor.matmul(ps[:], WkRes[:, dt_, t_out * P:(t_out + 1) * P],
                                         xb[:, dt_, :], start=(dt_ == 0), stop=(dt_ == NT - 1))
                    nc.vector.tensor_copy(kraw[:, t_out, :], ps[:])
                ck = csp.tile([P, NT // 2, SB], BF16, tag="cs_c")
                nc.gpsimd.dma_start(ck[:], cosK_r[:, :, sl])
                sk = csp.tile([P, NT // 2, SB], BF16, tag="cs_s")
                nc.gpsimd.dma_start(sk[:], sinK_r[:, :, sl])
                rope_block(KT[:, :, sl], kraw, ck, sk)
                # V rows for this block
                for sk_ in range(SB // P):
                    for dh in range(2):
                        ps = psB.tile([P, SB], F32, tag="psB")
                        for dt_ in range(NT):
                            nc.tensor.matmul(ps[:], xb[:, dt_, sk_ * P:(sk_ + 1) * P],
                                             WvRes[:, dt_, dh * SB:(dh + 1) * SB],
                                             start=(dt_ == 0), stop=(dt_ == NT - 1))
                        nc.scalar.copy(V[:, sb * (SB // P) + sk_, dh * SB:(dh + 1) * SB], ps[:])

                # ---------- wave B of the previous slot (PE overlap) --------
                if sb > 0:
                    pc, pqsl, pPT, pbc = prev
                    pvB = [psPV.tile([P, SB], F32, tag="pv", name=f"pvB{sb}_{j}")
                           for j in range(4)]
                    for v in range(pc):
                        for j in range(4):
                            nc.tensor.matmul(pvB[j][:, 0:QB],
                                             V[:, v, (4 + j) * P:(5 + j) * P],
                                             pPT[:, v, :], start=(v == 0),
                                             stop=(v == pc - 1))
                    for j in range(4):
                        nc.vector.tensor_mul(out=attnT[:, 4 + j, pqsl],
                                             in0=pvB[j][:, 0:QB], in1=pbc[:])

                # ---------- attention slot sb (wave A in-loop) ----------
                s = sb
                c = NVIS[s]
                qsl = slice(s * QB, (s + 1) * QB)
                sumacc = sap.tile([P, QB], F32R, tag="sa")
                PT = ptp.tile([P, 16, QB], BF16, tag="pts")
                pvA = [psPV.tile([P, SB], F32, tag="pv", name=f"pvA{s}_{j}")
                       for j in range(4)]
                for v in range(c):
                    ps = psB.tile([P, SB], F32, tag="psB")
                    for dt_ in range(NT):
                        nc.tensor.matmul(ps[:, 0:QB], KT[:, dt_, v * P:(v + 1) * P],
                                         QT[:, dt_, qsl], start=(dt_ == 0), stop=(dt_ == NT - 1))
                    nc.scalar.activation(PT[:, v, :], ps[:, 0:QB],
                                         mybir.ActivationFunctionType.Exp, scale=SCALE)
                    if v >= c - 4:
                        nc.vector.tensor_mul(out=PT[:, v, :], in0=PT[:, v, :],
                                             in1=maskst[:, 4 * s + (v - (c - 4)), :])
                    if v == 0:
                        nc.vector.tensor_copy(sumacc[:], PT[:, v, :])
                    else:
                        nc.vector.tensor_tensor(sumacc[:], sumacc[:], PT[:, v, :],
                                                mybir.AluOpType.add)
                    if v > 0:
                        for j in range(4):
                            nc.tensor.matmul(pvA[j][:, 0:QB],
                                             V[:, v - 1, j * P:(j + 1) * P], PT[:, v - 1, :],
                                             start=(v - 1 == 0), stop=False)
                for j in range(4):
                    nc.tensor.matmul(pvA[j][:, 0:QB],
                                     V[:, c - 1, j * P:(j + 1) * P], PT[:, c - 1, :],
                                     start=(c == 1), stop=True)
                # normalize: 1/rowsum broadcast via PE, scale wave-A chunks
                sums_ps = psB.tile([P, SB], F32, tag="psB")
                nc.tensor.matmul(sums_ps[0:1, 0:QB], ones_col[:], sumacc[:],
                                 start=True, stop=True)
                sumrow = smp.tile([1, QB], F32R, tag="sumrow")
                nc.scalar.copy(sumrow[:], sums_ps[0:1, 0:QB])
                bc_ps = psB.tile([P, SB], F32, tag="psB")
                nc.tensor.matmul(bc_ps[:, 0:QB], ones_row[:], sumrow[:],
                                 start=True, stop=True)
                bc_sums = bcp.tile([P, QB], F32R, tag="bcs")
                nc.scalar.copy(bc_sums[:], bc_ps[:, 0:QB])
                bc = bcp.tile([P, QB], F32, tag="bc")
                nc.vector.reciprocal(bc[:], bc_sums[:])
                for j in range(4):
                    nc.vector.tensor_mul(out=attnT[:, j, qsl],
                                         in0=pvA[j][:, 0:QB], in1=bc[:])
                prev = (c, qsl, PT, bc)

            # ---------- wave B of the last slot ----------
            pc, pqsl, pPT, pbc = prev
            pvB = [psPV.tile([P, SB], F32, tag="pv", name=f"pvBf_{j}")
                   for j in range(4)]
            for v in range(pc):
                for j in range(4):
                    nc.tensor.matmul(pvB[j][:, 0:QB],
                                     V[:, v, (4 + j) * P:(5 + j) * P],
                                     pPT[:, v, :], start=(v == 0), stop=(v == pc - 1))
            for j in range(4):
                nc.vector.tensor_mul(out=attnT[:, 4 + j, pqsl],
                                     in0=pvB[j][:, 0:QB], in1=pbc[:])

            # ---------- output projection ----------
            for qh in range(4):
                sl = slice(qh * QB, (qh + 1) * QB)
                ob = obp.tile([P, NT, QB], BF16, tag="ob")
                for oc in range(NT):
                    ps = psB.tile([P, SB], F32, tag="psB")
                    for dt_ in range(NT):
                        nc.tensor.matmul(ps[:, 0:QB], WoRes[:, dt_, oc * P:(oc + 1) * P],
                                         attnT[:, dt_, sl], start=(dt_ == 0), stop=(dt_ == NT - 1))
                    nc.scalar.copy(ob[:, oc, :], ps[:, 0:QB])
                    nc.sync.dma_start(outT[:, oc, sl], ob[:, oc, :])

    nc.finalize()
    return nc


def _host_inputs(x, Wq, Wk, Wv, Wo, token_positions):
    import ml_dtypes
    bf = ml_dtypes.bfloat16
    perm = np.concatenate([np.arange(0, D, 2), np.arange(1, D, 2)])
    WqTp = np.ascontiguousarray(Wq[perm].T).astype(bf)
    WkTp = np.ascontiguousarray(Wk[perm].T).astype(bf)
    WvT = np.ascontiguousarray(Wv.T).astype(bf)
    WoT = np.ascontiguousarray(Wo.T).astype(bf)
    inv_freq = (1.0 / (np.float32(THETA) **
                       (np.arange(0, D, 2, dtype=np.float32) / np.float32(D))))
    ones_col = np.ones((P, 1), np.float32)
    ones_row = np.ones((1, P), np.float32)

    in_maps, metas = [], []
    for b in range(B):
        xT = np.ascontiguousarray(x[b].T).astype(bf)           # [D, S]
        pos = token_positions[b].astype(np.float32)
        ang = (pos[None, :] * inv_freq[:, None]).astype(np.float32)  # [D/2, S]
        cosF = np.cos(ang)
        sinF = np.sin(ang)
        for h in range(2):
            blocks = BLOCKS[h]
            qcols = np.concatenate([np.arange(QB * bs, QB * (bs + 1))
                                    for bs in blocks])
            xTq = np.ascontiguousarray(xT[:, qcols])
            cosQ = np.ascontiguousarray(cosF[:, qcols]).astype(bf)
            sinQ = np.ascontiguousarray(sinF[:, qcols]).astype(bf)
            m = np.zeros((P, 16, QB), dtype=np.float32)
            for s, bs in enumerate(blocks):
                c = NVIS[s]
                q0 = QB * bs
                q_glob = q0 + np.arange(QB)
                for j in range(4):
                    v = c - 4 + j
                    k_glob = 128 * v + np.arange(P)
                    m[:, 4 * s + j, :] = (q_glob[None, :] >= k_glob[:, None])
            in_maps.append({
                "ones_col": ones_col, "ones_row": ones_row,
                "xT": xT, "xTq": xTq,
                "WqT": WqTp, "WkT": WkTp, "WvT": WvT, "WoT": WoT,
                "cosK": cosF.astype(bf), "sinK": sinF.astype(bf),
                "cosQ": cosQ, "sinQ": sinQ,
                "masks": m.astype(bf),
            })
            metas.append((b, qcols))
    return in_maps, metas


_NC_CACHE = {}


def kernel(x, Wq, Wk, Wv, Wo, token_positions):
    x = np.asarray(x); token_positions = np.asarray(token_positions)
    if "nc" not in _NC_CACHE:
        _NC_CACHE["nc"] = _build_program()
    nc = _NC_CACHE["nc"]
    in_maps, metas = _host_inputs(np.asarray(x), np.asarray(Wq), np.asarray(Wk),
                                  np.asarray(Wv), np.asarray(Wo), token_positions)
    res = run_bass_kernel_spmd(nc, in_maps, core_ids=list(range(8)))
    out = np.empty((B, S, D), dtype=np.float32)
    for (b, qcols), r in zip(metas, res.results):
        oT = np.asarray(r["outT"]).astype(np.float32)   # [P, NT, NQ]
        o = np.transpose(oT, (2, 1, 0)).reshape(NQ, D)
        out[b, qcols, :] = o
    return out
